# revision 21
# baseline (speedup 1.0000x reference)
"""BFP-quantized linear layer (BFLinear) for Trainium2, 8-core data-parallel.

Computes: out = bfp_q(x, 8, 16) @ bfp_q(w, 8, 16).T + bias
  where bfp_q groups 16 contiguous elements along the feature axis, shares
  exponent e = floor(log2(max|g|)), rounds mantissas to `bit` bits (RNE) and
  clips to [-2^(bit-1), 2^(bit-1)-1].

On-device math per 512-row chunk (tile [128, 2048], rows on partitions):
  gmax   = max|group|                   (DVE reduce, f32 - exact exponent)
  pow2e  = bits(gmax) & EXPMASK         (DVE TS int)
  recipB = bits^EXPMASK + (bit-2)<<23   (DVE TS int = f32 2^(bit-1-e))
  pairs  = recipB,scab duplicated 2x    (ACT dup-read copies -> bf16 "pairs"
           packing; lets the DVE TT read the group-broadcast operand as
           coalesced 4B pairs: 1460ns vs 2278ns per [128,2048] measured)
  v      = x * recipB -> bf16           (TT, exact pow2 scaling; Pool)
  u      = clamp(v, -128, 127)          (TS min/max bf16, DVE; plain integer
           bounds are round-then-clip equivalent: see note below)
  t      = (u + C) + (-C) -> bf16 ints  (TS add/add, DVE; C=1.5*2^23 RNE trick)
  xq     = t * scab -> bf16             (TT pairs, split DVE/Pool)
  xqT    = xbar DMA transpose of xq     (hardware crossbar, 2-byte dtypes,
           out[do,di,m] = in[m, di*128+do]; kills PE transposes + PSUM copy)
  out    = xq @ wq.T via 4x4 accumulating PE matmuls into PSUM f32
  out_sb = PSUM -> SBUF bf16            (ACT)
Bias is added on the host during the bf16->f32 upcast (exact f32 add), saving
the PE bias-seeding matmuls. Output written bf16 (halves HBM write traffic).

v in bf16: x*2^k rounds to bf16 once before the integer RNE; double rounding
perturbs ~4% of elements by 1 int step (measured rel err 5.8e-3 max-scaled vs
2.6e-3 for the f32 path, both far under the 2e-2 gate). Set VDT="f32" to
fall back to the exact-reference path (costs ~8us).

Clamp-then-round with integer bounds (-128.0, 127.0) equals the reference's
round-then-clip for every case incl. ties (127.5 -> min gives 127; reference
RNE(127.5)=128 -> clip 127).

Engine assignment (measured ns per [128,2048] op): DVE reduce 2278 + smalls
438 + clamp 830 + round 830 + scale_dve-part; Pool mult 4639 + scale rest;
ACT pairs 2x~600 + outcopy 2x1114. Scale split is range-based so every chunk
is identical. All DMA dispatched from the sync queue.

Hardware op-shape rules learned from traces (violating any costs 2-25x):
  - scalar_tensor_tensor is DVE-only; tensor_scalar on Pool only with min/max;
    no negative int immediates on TS; f32->bf16 TS writes via (add,add) on DVE.
  - TT with [0,16] broadcast AP runs 1 elem/cycle regardless of dtype; pairs
    packing ([1,2] inner) recovers ~1.55x for bf16.
  - DVE/Pool reads from PSUM run at half rate; Pool cannot read PSUM at all
    (NEFF codegen failure). PSUM->SBUF conversion copies belong on ACT.
  - f32-out TT from bf16 inputs runs at half rate (2744 vs 1457).
"""

import os
import sys

import numpy as np

for _p in ("/opt/trn_rl_repo",):
    if _p not in sys.path and os.path.isdir(_p):
        sys.path.append(_p)

N_CORES = 8

# DVE and GpSimd share one SBUF port (concurrent ops degrade to the
# combined serial rate), so all elementwise work runs on DVE at clean rates
# and GpSimd stays idle. ACT (own port) does convert/pairs/outcopy.
CFG = {
    "vdt": "bf16",     # v/u/t dtype: bf16 (fast, double-round) | f32 (exact)
    "mult": "v",
    "clamp": "g",
    "round": "v",
    "scale_dve_groups": 128,  # of 128 groups/chunk: DVE share of scale TT
    "outcopy": "a",
}

_CACHE = {}


def _eng(nc, which, idx=0):
    s = {"v": nc.vector, "g": nc.gpsimd, "a": nc.scalar}
    return s[which[idx % len(which)]]


def _bcast_group_ap(t, G, sz):
    """AP reading tile t[P, G] as [P, G, sz] with the last dim broadcast."""
    import concourse.bass as bass

    ap = t.ap.copy()
    ap.append([0, sz])
    return bass.AP(tensor=t.tensor, offset=t.offset, ap=ap)


def _pairs_ap(t, g0, ng):
    """Read pairs-packed tile t[P, 2G] (each group value duplicated) as
    [P, ng, 8, 2] covering groups [g0, g0+ng): coalesced 4B pair reads."""
    import concourse.bass as bass

    view = t[:, 2 * g0 : 2 * (g0 + ng)]
    ap = view.ap.copy()[:1]
    ap.append([2, ng])
    ap.append([0, 8])
    ap.append([1, 2])
    return bass.AP(tensor=view.tensor, offset=view.offset, ap=ap)


def _build(nrows, K, O, x_bit, w_bit, x_sz, w_sz, cfg=None):
    import concourse.bacc as bacc
    import concourse.bass as bass  # noqa: F401
    import concourse.mybir as mybir
    import concourse.tile as tile
    from concourse.masks import make_identity

    cfg = dict(CFG, **(cfg or {}))
    f32 = mybir.dt.float32
    bf16 = mybir.dt.bfloat16
    i32 = mybir.dt.int32
    A = mybir.AluOpType

    assert x_bit == w_bit == 8 and x_sz == w_sz == 16
    bit, sz = x_bit, x_sz
    EM = 0x7F800000
    KADD = (bit - 2) << 23
    # v = x*2^(1-e) lives on the 2^-(bit-2) grid: C and the clamp bounds are
    # scaled by 2^-(bit-2); xq = t*2^e comes out 2x the reference, fixed by a
    # 0.5 scale immediate in the outcopy (all exact powers of two).
    C = float(np.float32(1.5 * 2.0 ** (23 - (bit - 2))))
    qhi = float((2 ** (bit - 1) - 1) * 2.0 ** (-(bit - 2)))
    qlo = float(-(2 ** (bit - 1)) * 2.0 ** (-(bit - 2)))

    P = 128
    RPC = 512
    assert nrows % RPC == 0
    n_chunks = nrows // RPC
    FB = RPC // P            # 4 row-blocks per chunk
    F = FB * K               # 2048 free columns per chunk
    G = F // sz              # 128 groups per chunk
    KB = K // P              # 4 k-blocks
    OB = O // P              # 4 o-blocks
    GW = K // sz             # 32 groups per weight row-tile
    vdt = bf16 if cfg["vdt"] == "bf16" else f32
    sdg = cfg["scale_dve_groups"]

    nc = bacc.Bacc("TRN2", debug=False)
    x_d = nc.dram_tensor("x", (nrows, K), f32, kind="ExternalInput").ap()
    wt_d = nc.dram_tensor("wqt", (K, O), bf16, kind="ExternalInput").ap()
    o_d = nc.dram_tensor("out", (nrows, O), bf16, kind="ExternalOutput").ap()

    with tile.TileContext(nc) as tc:
        with (
            tc.tile_pool(name="const", bufs=1) as constp,
            tc.tile_pool(name="xraw", bufs=4) as xraw,
            tc.tile_pool(name="sml", bufs=4) as sml,
            tc.tile_pool(name="xb", bufs=4) as xbp,
            tc.tile_pool(name="prs", bufs=4) as prs,
            tc.tile_pool(name="v", bufs=3) as vp,
            tc.tile_pool(name="u", bufs=2) as up,
            tc.tile_pool(name="t", bufs=2) as tp,
            tc.tile_pool(name="xq", bufs=4) as xqp,
            tc.tile_pool(name="xqT", bufs=4) as xqTp,
            tc.tile_pool(name="osb", bufs=4) as osb,
            tc.tile_pool(name="psO", bufs=2, space="PSUM") as psO,
            tc.tile_pool(name="psT", bufs=2, space="PSUM") as psT,
        ):
            p2s = constp.tile([P, 1], f32)
            nc.vector.memset(p2s, float(2.0 ** (-(bit - 1))))
            r2s = constp.tile([P, 1], f32)
            nc.vector.memset(r2s, float(2.0 ** (bit - 2)))
            ident = constp.tile([P, P], bf16)
            make_identity(nc, ident)

            # ---- software-pipelined main loop ----
            st = {}

            def dma_in(i):
                x_raw = xraw.tile([P, FB, K], f32, tag="x_raw")
                src = x_d[i * RPC : (i + 1) * RPC, :].rearrange(
                    "(f p) k -> p f k", p=P
                )
                nc.sync.dma_start(out=x_raw, in_=src)
                st[i] = {"x": x_raw}

            def quant_a(i):
                s = st[i]
                xt = s["x"].rearrange("p f k -> p (f k)")
                s["xt"] = xt
                gmx = sml.tile([P, G], f32, tag="gmx")
                nc.vector.tensor_reduce(
                    out=gmx,
                    in_=xt.rearrange("p (g s) -> p g s", s=sz),
                    axis=mybir.AxisListType.X,
                    op=A.max,
                    apply_absolute_value=True,
                )
                rb0 = sml.tile([P, G], i32, tag="rb0")
                nc.vector.tensor_scalar(
                    out=rb0, in0=gmx.bitcast(i32), scalar1=EM, scalar2=EM,
                    op0=A.bitwise_and, op1=A.bitwise_xor,
                )
                pe = sml.tile([P, G], i32, tag="pe")
                nc.vector.tensor_scalar(
                    out=pe, in0=rb0, scalar1=EM, scalar2=None,
                    op0=A.bitwise_xor,
                )
                # x -> bf16 on ACT (deep-skewed off the critical chain);
                # the mult then runs 1460ns (bf16 pairs) vs 2291ns (f32 in0)
                xb = xbp.tile([P, F], bf16, tag="xb")
                nc.scalar.copy(xb, xt)
                s["xb"] = xb
                rbp = prs.tile([P, 2 * G], bf16, tag="rbp")
                nc.scalar.copy(rbp, _bcast_group_ap(rb0.bitcast(f32), G, 2))
                scp = prs.tile([P, 2 * G], bf16, tag="scp")
                nc.scalar.copy(scp, _bcast_group_ap(pe.bitcast(f32), G, 2))
                s["rbp"], s["scp"] = rbp, scp

            def quant_b(i):
                s = st.pop(i)
                v = vp.tile([P, F], vdt, tag="v")
                _eng(nc, cfg["mult"], i).tensor_tensor(
                    out=v, in0=s["xb"], in1=_pairs_ap(s["rbp"], 0, G), op=A.mult,
                )
                u = up.tile([P, F], vdt, tag="u")
                _eng(nc, cfg["clamp"], i).tensor_scalar(
                    out=u, in0=v, scalar1=qhi + 0.0, scalar2=qlo + 0.0,
                    op0=A.min, op1=A.max,
                )
                t = tp.tile([P, F], bf16, tag="t")
                _eng(nc, cfg["round"], i).tensor_scalar(
                    out=t, in0=u, scalar1=C, scalar2=-C, op0=A.add, op1=A.add,
                )
                xq = xqp.tile([P, F], bf16, tag="xq")
                if sdg > 0:
                    nc.vector.tensor_tensor(
                        out=xq[:, : sdg * sz], in0=t[:, : sdg * sz],
                        in1=_pairs_ap(s["scp"], 0, sdg), op=A.mult,
                    )
                if sdg < G:
                    nc.gpsimd.tensor_tensor(
                        out=xq[:, sdg * sz :], in0=t[:, sdg * sz :],
                        in1=_pairs_ap(s["scp"], sdg, G - sdg), op=A.mult,
                    )
                xq_nat = xq.rearrange("p (f c q) -> p f c q", f=FB, c=KB)
                ptT = psT.tile([P, FB * KB, P], bf16, tag="ptT")
                for fb in range(FB):
                    for kb in range(KB):
                        nc.tensor.transpose(
                            ptT[:, fb * KB + kb, :], xq_nat[:, fb, kb], ident
                        )
                xqT = xqTp.tile([P, FB * KB, P], bf16, tag="xqT")
                nc.scalar.copy(xqT, ptT)
                st[i] = {"xqT": xqT}

            def mm_out(i):
                s = st.pop(i)
                xqT = s["xqT"]
                for fp in range(FB // 2):
                    po = psO.tile([P, 2, O], f32, tag="po")
                    for g in range(2):
                        fb = fp * 2 + g
                        for kb in range(KB):
                            nc.tensor.matmul(
                                po[:, g, :],
                                lhsT=xqT[:, fb * KB + kb, :],
                                rhs=wqT[kb],
                                start=(kb == 0),
                                stop=(kb == KB - 1),
                            )
                    out_sb = osb.tile([P, 2, O], bf16, tag="out_sb")
                    nc.scalar.activation(
                        out_sb, po, mybir.ActivationFunctionType.Copy, scale=0.5
                    )
                    rr = i * RPC + fp * 2 * P
                    dst = o_d[rr : rr + 2 * P, :].rearrange("(f p) k -> p f k", p=P)
                    nc.sync.dma_start(out=dst, in_=out_sb)

            dma_in(0)
            # weights after x0: x0's input is on the critical path, the wqT
            # tiles aren't needed until the first matmul (~25us in)
            wqT = []
            for kb in range(KB):
                wt = constp.tile([P, O], bf16, tag=f"wqT{kb}", bufs=KB)
                nc.sync.dma_start(out=wt, in_=wt_d[kb * P : (kb + 1) * P, :])
                wqT.append(wt)
            for j in range(1, min(3, n_chunks)):
                dma_in(j)
            quant_a(0)
            if n_chunks > 1:
                quant_a(1)
            for i in range(n_chunks):
                if i + 3 < n_chunks:
                    dma_in(i + 3)
                quant_b(i)
                if i + 2 < n_chunks:
                    quant_a(i + 2)
                if i >= 1:
                    mm_out(i - 1)
                if i == n_chunks - 1:
                    mm_out(i)
    nc.compile()
    return nc


def _get_program(nrows, K, O, x_bit, w_bit, x_sz, w_sz):
    key = (nrows, K, O, x_bit, w_bit, x_sz, w_sz)
    if key not in _CACHE:
        _CACHE[key] = _build(nrows, K, O, x_bit, w_bit, x_sz, w_sz)
    return _CACHE[key]


def _host_bfp_quantize(w, bit, sz):
    """Reference bfp_quantize in float32 numpy (np.round is RNE like jnp)."""
    g = w.reshape(-1, sz)
    maxabs = np.max(np.abs(g), axis=1, keepdims=True)
    e = np.floor(np.log2(np.maximum(maxabs, np.float32(1e-38), dtype=np.float32)))
    scale = np.exp2(e - (bit - 1), dtype=np.float32)
    qmax = np.float32(2.0 ** (bit - 1) - 1.0)
    q = np.clip(np.round(g / scale), -qmax - 1.0, qmax) * scale
    return q.reshape(w.shape).astype(np.float32)


def kernel(input, weight, bias, i_bit, i_sz, w_bit, w_sz):
    import ml_dtypes
    from concourse.bass_utils import run_bass_kernel_spmd

    x = np.ascontiguousarray(np.asarray(input, dtype=np.float32))
    w = np.ascontiguousarray(np.asarray(weight, dtype=np.float32))
    b = np.asarray(bias, dtype=np.float32).reshape(1, -1)
    i_bit, i_sz, w_bit, w_sz = int(i_bit), int(i_sz), int(w_bit), int(w_sz)

    N, K = x.shape
    O = w.shape[0]
    assert N % N_CORES == 0
    shard = N // N_CORES

    # wq values are int*2^e, exactly representable in bf16
    wqt = np.ascontiguousarray(
        _host_bfp_quantize(w, w_bit, w_sz).T
    ).astype(ml_dtypes.bfloat16)

    nc = _get_program(shard, K, O, i_bit, w_bit, i_sz, w_sz)
    in_maps = [
        {"x": x[i * shard : (i + 1) * shard], "wqt": wqt} for i in range(N_CORES)
    ]
    res = run_bass_kernel_spmd(nc, in_maps, list(range(N_CORES)))
    out = np.empty((N, O), dtype=np.float32)
    for i, r in enumerate(res.results):
        np.add(
            np.asarray(r["out"]).astype(np.float32), b,
            out=out[i * shard : (i + 1) * shard],
        )
    return out


# revision 22
# speedup vs baseline: 1.0705x; 1.0705x over previous
"""BFP-quantized linear layer (BFLinear) for Trainium2, 8-core data-parallel.

Computes: out = bfp_q(x, 8, 16) @ bfp_q(w, 8, 16).T + bias
  where bfp_q groups 16 contiguous elements along the feature axis, shares
  exponent e = floor(log2(max|g|)), rounds mantissas to `bit` bits (RNE) and
  clips to [-2^(bit-1), 2^(bit-1)-1].

On-device math per 512-row chunk (tile [128, 2048], rows on partitions):
  gmax   = max|group|                   (DVE reduce, f32 - exact exponent)
  pow2e  = bits(gmax) & EXPMASK         (DVE TS int)
  recipB = bits^EXPMASK + (bit-2)<<23   (DVE TS int = f32 2^(bit-1-e))
  pairs  = recipB,scab duplicated 2x    (ACT dup-read copies -> bf16 "pairs"
           packing; lets the DVE TT read the group-broadcast operand as
           coalesced 4B pairs: 1460ns vs 2278ns per [128,2048] measured)
  v      = x * recipB -> bf16           (TT, exact pow2 scaling; Pool)
  u      = clamp(v, -128, 127)          (TS min/max bf16, DVE; plain integer
           bounds are round-then-clip equivalent: see note below)
  t      = (u + C) + (-C) -> bf16 ints  (TS add/add, DVE; C=1.5*2^23 RNE trick)
  xq     = t * scab -> bf16             (TT pairs, split DVE/Pool)
  xqT    = xbar DMA transpose of xq     (hardware crossbar, 2-byte dtypes,
           out[do,di,m] = in[m, di*128+do]; kills PE transposes + PSUM copy)
  out    = xq @ wq.T via 4x4 accumulating PE matmuls into PSUM f32
  out_sb = PSUM -> SBUF bf16            (ACT)
Bias is added on the host during the bf16->f32 upcast (exact f32 add), saving
the PE bias-seeding matmuls. Output written bf16 (halves HBM write traffic).

v in bf16: x*2^k rounds to bf16 once before the integer RNE; double rounding
perturbs ~4% of elements by 1 int step (measured rel err 5.8e-3 max-scaled vs
2.6e-3 for the f32 path, both far under the 2e-2 gate). Set VDT="f32" to
fall back to the exact-reference path (costs ~8us).

Clamp-then-round with integer bounds (-128.0, 127.0) equals the reference's
round-then-clip for every case incl. ties (127.5 -> min gives 127; reference
RNE(127.5)=128 -> clip 127).

Engine assignment (measured ns per [128,2048] op): DVE reduce 2278 + smalls
438 + clamp 830 + round 830 + scale_dve-part; Pool mult 4639 + scale rest;
ACT pairs 2x~600 + outcopy 2x1114. Scale split is range-based so every chunk
is identical. All DMA dispatched from the sync queue.

Hardware op-shape rules learned from traces (violating any costs 2-25x):
  - scalar_tensor_tensor is DVE-only; tensor_scalar on Pool only with min/max;
    no negative int immediates on TS; f32->bf16 TS writes via (add,add) on DVE.
  - TT with [0,16] broadcast AP runs 1 elem/cycle regardless of dtype; pairs
    packing ([1,2] inner) recovers ~1.55x for bf16.
  - DVE/Pool reads from PSUM run at half rate; Pool cannot read PSUM at all
    (NEFF codegen failure). PSUM->SBUF conversion copies belong on ACT.
  - f32-out TT from bf16 inputs runs at half rate (2744 vs 1457).
"""

import os
import sys

import numpy as np

for _p in ("/opt/trn_rl_repo",):
    if _p not in sys.path and os.path.isdir(_p):
        sys.path.append(_p)

N_CORES = 8

# DVE and GpSimd share one SBUF port (concurrent ops degrade to the
# combined serial rate), so all elementwise work runs on DVE at clean rates
# and GpSimd stays idle. ACT (own port) does convert/pairs/outcopy.
CFG = {
    "vdt": "bf16",     # v/u/t dtype: bf16 (fast, double-round) | f32 (exact)
    "mult": "v",
    "clamp": "g",
    "round": "v",
    "scale_dve_groups": 128,  # of 128 groups/chunk: DVE share of scale TT
    "outcopy": "a",
}

_CACHE = {}


def _eng(nc, which, idx=0):
    s = {"v": nc.vector, "g": nc.gpsimd, "a": nc.scalar}
    return s[which[idx % len(which)]]


def _bcast_group_ap(t, G, sz):
    """AP reading tile t[P, G] as [P, G, sz] with the last dim broadcast."""
    import concourse.bass as bass

    ap = t.ap.copy()
    ap.append([0, sz])
    return bass.AP(tensor=t.tensor, offset=t.offset, ap=ap)


def _pairs_ap(t, g0, ng):
    """Read pairs-packed tile t[P, 2G] (each group value duplicated) as
    [P, ng, 8, 2] covering groups [g0, g0+ng): coalesced 4B pair reads."""
    import concourse.bass as bass

    view = t[:, 2 * g0 : 2 * (g0 + ng)]
    ap = view.ap.copy()[:1]
    ap.append([2, ng])
    ap.append([0, 8])
    ap.append([1, 2])
    return bass.AP(tensor=view.tensor, offset=view.offset, ap=ap)


def _build(nrows, K, O, x_bit, w_bit, x_sz, w_sz, cfg=None):
    import concourse.bacc as bacc
    import concourse.bass as bass  # noqa: F401
    import concourse.mybir as mybir
    import concourse.tile as tile
    from concourse.masks import make_identity

    cfg = dict(CFG, **(cfg or {}))
    f32 = mybir.dt.float32
    bf16 = mybir.dt.bfloat16
    i32 = mybir.dt.int32
    A = mybir.AluOpType

    assert x_bit == w_bit == 8 and x_sz == w_sz == 16
    bit, sz = x_bit, x_sz
    EM = 0x7F800000
    KADD = (bit - 2) << 23
    # v = x*2^(1-e) lives on the 2^-(bit-2) grid: C and the clamp bounds are
    # scaled by 2^-(bit-2); xq = t*2^e comes out 2x the reference, fixed by a
    # 0.5 scale immediate in the outcopy (all exact powers of two).
    C = float(np.float32(1.5 * 2.0 ** (23 - (bit - 2))))
    qhi = float((2 ** (bit - 1) - 1) * 2.0 ** (-(bit - 2)))
    qlo = float(-(2 ** (bit - 1)) * 2.0 ** (-(bit - 2)))

    P = 128
    RPC = 512
    assert nrows % RPC == 0
    n_chunks = nrows // RPC
    FB = RPC // P            # 4 row-blocks per chunk
    F = FB * K               # 2048 free columns per chunk
    G = F // sz              # 128 groups per chunk
    KB = K // P              # 4 k-blocks
    OB = O // P              # 4 o-blocks
    GW = K // sz             # 32 groups per weight row-tile
    vdt = bf16 if cfg["vdt"] == "bf16" else f32
    sdg = cfg["scale_dve_groups"]

    nc = bacc.Bacc("TRN2", debug=False)
    x_d = nc.dram_tensor("x", (nrows, K), f32, kind="ExternalInput").ap()
    wt_d = nc.dram_tensor("wqt", (K, O), bf16, kind="ExternalInput").ap()
    o_d = nc.dram_tensor("out", (nrows, O), bf16, kind="ExternalOutput").ap()

    with tile.TileContext(nc) as tc:
        with (
            tc.tile_pool(name="const", bufs=1) as constp,
            tc.tile_pool(name="xraw", bufs=4) as xraw,
            tc.tile_pool(name="sml", bufs=4) as sml,
            tc.tile_pool(name="xb", bufs=4) as xbp,
            tc.tile_pool(name="prs", bufs=4) as prs,
            tc.tile_pool(name="v", bufs=3) as vp,
            tc.tile_pool(name="u", bufs=2) as up,
            tc.tile_pool(name="t", bufs=2) as tp,
            tc.tile_pool(name="xq", bufs=4) as xqp,
            tc.tile_pool(name="xqT", bufs=4) as xqTp,
            tc.tile_pool(name="osb", bufs=4) as osb,
            tc.tile_pool(name="psO", bufs=2, space="PSUM") as psO,
            tc.tile_pool(name="psT", bufs=2, space="PSUM") as psT,
        ):
            p2s = constp.tile([P, 1], f32)
            nc.vector.memset(p2s, float(2.0 ** (-(bit - 1))))
            r2s = constp.tile([P, 1], f32)
            nc.vector.memset(r2s, float(2.0 ** (bit - 2)))
            ident = constp.tile([P, P], bf16)
            make_identity(nc, ident)

            # ---- software-pipelined main loop ----
            st = {}

            def dma_in(i):
                x_raw = xraw.tile([P, FB, K], f32, tag="x_raw")
                src = x_d[i * RPC : (i + 1) * RPC, :].rearrange(
                    "(f p) k -> p f k", p=P
                )
                nc.sync.dma_start(out=x_raw, in_=src)
                st[i] = {"x": x_raw}

            def quant_a(i):
                s = st[i]
                xt = s["x"].rearrange("p f k -> p (f k)")
                s["xt"] = xt
                gmx = sml.tile([P, G], f32, tag="gmx")
                nc.vector.tensor_reduce(
                    out=gmx,
                    in_=xt.rearrange("p (g s) -> p g s", s=sz),
                    axis=mybir.AxisListType.X,
                    op=A.max,
                    apply_absolute_value=True,
                )
                rb0 = sml.tile([P, G], i32, tag="rb0")
                nc.vector.tensor_scalar(
                    out=rb0, in0=gmx.bitcast(i32), scalar1=EM, scalar2=EM,
                    op0=A.bitwise_and, op1=A.bitwise_xor,
                )
                pe = sml.tile([P, G], i32, tag="pe")
                nc.vector.tensor_scalar(
                    out=pe, in0=rb0, scalar1=EM, scalar2=None,
                    op0=A.bitwise_xor,
                )
                # x -> bf16 on ACT (deep-skewed off the critical chain);
                # the mult then runs 1460ns (bf16 pairs) vs 2291ns (f32 in0)
                xb = xbp.tile([P, F], bf16, tag="xb")
                nc.scalar.copy(xb, xt)
                s["xb"] = xb
                rbp = prs.tile([P, 2 * G], bf16, tag="rbp")
                nc.vector.tensor_copy(
                    out=rbp, in_=_bcast_group_ap(rb0.bitcast(f32), G, 2)
                )
                scp = prs.tile([P, 2 * G], bf16, tag="scp")
                nc.vector.tensor_copy(
                    out=scp, in_=_bcast_group_ap(pe.bitcast(f32), G, 2)
                )
                s["rbp"], s["scp"] = rbp, scp

            def quant_b(i):
                s = st.pop(i)
                v = vp.tile([P, F], vdt, tag="v")
                _eng(nc, cfg["mult"], i).tensor_tensor(
                    out=v, in0=s["xb"], in1=_pairs_ap(s["rbp"], 0, G), op=A.mult,
                )
                u = up.tile([P, F], vdt, tag="u")
                _eng(nc, cfg["clamp"], i).tensor_scalar(
                    out=u, in0=v, scalar1=qhi + 0.0, scalar2=qlo + 0.0,
                    op0=A.min, op1=A.max,
                )
                t = tp.tile([P, F], bf16, tag="t")
                _eng(nc, cfg["round"], i).tensor_scalar(
                    out=t, in0=u, scalar1=C, scalar2=-C, op0=A.add, op1=A.add,
                )
                xq = xqp.tile([P, F], bf16, tag="xq")
                if sdg > 0:
                    nc.vector.tensor_tensor(
                        out=xq[:, : sdg * sz], in0=t[:, : sdg * sz],
                        in1=_pairs_ap(s["scp"], 0, sdg), op=A.mult,
                    )
                if sdg < G:
                    nc.gpsimd.tensor_tensor(
                        out=xq[:, sdg * sz :], in0=t[:, sdg * sz :],
                        in1=_pairs_ap(s["scp"], sdg, G - sdg), op=A.mult,
                    )
                xq_nat = xq.rearrange("p (f c q) -> p f c q", f=FB, c=KB)
                ptT = psT.tile([P, FB * KB, P], bf16, tag="ptT")
                for fb in range(FB):
                    for kb in range(KB):
                        nc.tensor.transpose(
                            ptT[:, fb * KB + kb, :], xq_nat[:, fb, kb], ident
                        )
                xqT = xqTp.tile([P, FB * KB, P], bf16, tag="xqT")
                nc.scalar.copy(xqT, ptT)
                st[i] = {"xqT": xqT}

            def mm_out(i):
                s = st.pop(i)
                xqT = s["xqT"]
                for fp in range(FB // 2):
                    po = psO.tile([P, 2, O], f32, tag="po")
                    for g in range(2):
                        fb = fp * 2 + g
                        for kb in range(KB):
                            nc.tensor.matmul(
                                po[:, g, :],
                                lhsT=xqT[:, fb * KB + kb, :],
                                rhs=wqT[kb],
                                start=(kb == 0),
                                stop=(kb == KB - 1),
                            )
                    out_sb = osb.tile([P, 2, O], bf16, tag="out_sb")
                    nc.scalar.activation(
                        out_sb, po, mybir.ActivationFunctionType.Copy, scale=0.5
                    )
                    rr = i * RPC + fp * 2 * P
                    dst = o_d[rr : rr + 2 * P, :].rearrange("(f p) k -> p f k", p=P)
                    nc.sync.dma_start(out=dst, in_=out_sb)

            dma_in(0)
            # weights after x0: x0's input is on the critical path, the wqT
            # tiles aren't needed until the first matmul (~25us in)
            wqT = []
            for kb in range(KB):
                wt = constp.tile([P, O], bf16, tag=f"wqT{kb}", bufs=KB)
                nc.sync.dma_start(out=wt, in_=wt_d[kb * P : (kb + 1) * P, :])
                wqT.append(wt)
            for j in range(1, min(3, n_chunks)):
                dma_in(j)
            quant_a(0)
            if n_chunks > 1:
                quant_a(1)
            for i in range(n_chunks):
                if i + 3 < n_chunks:
                    dma_in(i + 3)
                quant_b(i)
                if i + 2 < n_chunks:
                    quant_a(i + 2)
                if i >= 1:
                    mm_out(i - 1)
                if i == n_chunks - 1:
                    mm_out(i)
    nc.compile()
    return nc


def _get_program(nrows, K, O, x_bit, w_bit, x_sz, w_sz):
    key = (nrows, K, O, x_bit, w_bit, x_sz, w_sz)
    if key not in _CACHE:
        _CACHE[key] = _build(nrows, K, O, x_bit, w_bit, x_sz, w_sz)
    return _CACHE[key]


def _host_bfp_quantize(w, bit, sz):
    """Reference bfp_quantize in float32 numpy (np.round is RNE like jnp)."""
    g = w.reshape(-1, sz)
    maxabs = np.max(np.abs(g), axis=1, keepdims=True)
    e = np.floor(np.log2(np.maximum(maxabs, np.float32(1e-38), dtype=np.float32)))
    scale = np.exp2(e - (bit - 1), dtype=np.float32)
    qmax = np.float32(2.0 ** (bit - 1) - 1.0)
    q = np.clip(np.round(g / scale), -qmax - 1.0, qmax) * scale
    return q.reshape(w.shape).astype(np.float32)


def kernel(input, weight, bias, i_bit, i_sz, w_bit, w_sz):
    import ml_dtypes
    from concourse.bass_utils import run_bass_kernel_spmd

    x = np.ascontiguousarray(np.asarray(input, dtype=np.float32))
    w = np.ascontiguousarray(np.asarray(weight, dtype=np.float32))
    b = np.asarray(bias, dtype=np.float32).reshape(1, -1)
    i_bit, i_sz, w_bit, w_sz = int(i_bit), int(i_sz), int(w_bit), int(w_sz)

    N, K = x.shape
    O = w.shape[0]
    assert N % N_CORES == 0
    shard = N // N_CORES

    # wq values are int*2^e, exactly representable in bf16
    wqt = np.ascontiguousarray(
        _host_bfp_quantize(w, w_bit, w_sz).T
    ).astype(ml_dtypes.bfloat16)

    nc = _get_program(shard, K, O, i_bit, w_bit, i_sz, w_sz)
    in_maps = [
        {"x": x[i * shard : (i + 1) * shard], "wqt": wqt} for i in range(N_CORES)
    ]
    res = run_bass_kernel_spmd(nc, in_maps, list(range(N_CORES)))
    out = np.empty((N, O), dtype=np.float32)
    for i, r in enumerate(res.results):
        np.add(
            np.asarray(r["out"]).astype(np.float32), b,
            out=out[i * shard : (i + 1) * shard],
        )
    return out
